# revision 1
# baseline (speedup 1.0000x reference)
"""EWMA predictor (sliding-window variance, exponentially weighted sum) on 8 trn2 cores.

Math: for j in [0, L): window_j = x[j : j+128], weight ff^(L-1-j),
result = norm * sum_j ff^(L-1-j) * var(window_j, ddof=1),
norm = (1-ff)/(1-ff^L), ff = sigmoid(raw_forgetting_factor).

Sharding: windows split over 8 cores x 128 partitions; partition p of core c
owns the 512 consecutive windows starting at base_c + 512*p and loads the 639
input elements covering them (halo overlap, contiguous per partition). The
per-core input tile carries ff and the per-partition combine coefficients
c_p = ff^i0(c,p)/127 in two extra trailing columns, so the input DMAs load
everything; the input DMA is split into column halves across the SP and ACT
HWDGE rings (a single full-width descriptor set measured ~10x slower).

Per-core device program (vector + scalar + PE engines):
  s1[t], s2[t]: sliding 128-window sums of x and x^2 via tensor_tensor_scan
                recurrence s[t] = (x[t+127] + s[t-1]) - x[t-1]
  d[t] = s2 - s1^2/128 = 127 * var
  e[t] = ff*e[t-1] + d[t]  (scan, ff read via stride-0 broadcast AP)
  contrib[p] = c_p * e[511]; PE matmul against const ones reduces over
  partitions to a single fp32 scalar, copied PSUM->SBUF and DMA'd out as a
  4-byte single-descriptor write (a [128,1] out = 128 descriptors measured
  ~6.4us vs ~free for 1 descriptor).
Host sums the 8 core scalars and applies norm in float64.

build_nc(reps=N) unrolls the body N times with serialized iterations — used
only for wall-clock loop timing (see bench_loop.py); the product kernel uses
reps=1.
"""

import numpy as np

import concourse.bass as bass
import concourse.mybir as mybir
from concourse.bass_utils import run_bass_kernel_spmd

L = 524288          # look-back windows
W = 128             # variance window length
N = L + W           # input length
NCORES = 8
WIN_PER_CORE = L // NCORES      # 65536
RUN = WIN_PER_CORE // 128       # 512 windows per partition
COLS = RUN + W - 1              # 639 input elems per partition
XTW = COLS + 2                  # + ff column + coeff column

_NC_CACHE = {}


def plan_run(ff64: float) -> int:
    """Windows-per-partition for the adaptive program.

    Weights ff^i are EXACTLY zero in fp32 (past subnormals) once
    i > 104/|ln ff|, so the reference's own terms there are zeros and windows
    beyond that cannot affect any output bit. Keep a >=1024-window margin,
    round the 1024*run window count up to a power-of-two run, clamp to
    [8, 512]; run=512 is the exact full computation (all L windows).
    """
    lnff = np.log(np.float64(ff64))
    if not (lnff < -1e-9):
        return RUN
    k_needed = 104.0 / (-lnff)
    run_min = int(np.ceil((k_needed + 1024.0) / 1024.0))
    run = 8
    while run < run_min:
        run *= 2
    return min(run, RUN)


def build_nc(reps: int = 1, run: int = RUN, small: bool | None = None) -> bass.Bass:
    """Per-core program. run=windows/partition. For small runs (<=64) the
    whole chain lives on the DVE (op bodies are tiny, so cross-engine
    semaphore hops cost more than the ACT offload saves, and with no
    activations at all the act-table load disappears); for large runs the
    squares run on the ACT engine overlapping the DVE scans."""
    cols = run + W - 1
    xtw = cols + 2
    if small is None:
        # The cost model favors the ACT-overlapped split chain at every run
        # size (5729 vs 5999 ns at run=8); the all-DVE path is kept for
        # experiments only.
        small = False
    nc = bass.Bass(trn_type="TRN2")
    f32 = mybir.dt.float32
    A = mybir.AluOpType
    xt = nc.declare_dram_parameter("xt", [128, xtw], f32, isOutput=False)
    acc = nc.declare_dram_parameter("acc", [1, 1], f32, isOutput=True)
    HALF = xtw // 2

    with (
        nc.sbuf_tensor([128, xtw], f32) as XT,
        nc.sbuf_tensor([128, cols], f32) as X2,
        nc.sbuf_tensor([128, run], f32) as S1,
        nc.sbuf_tensor([128, run], f32) as S2,
        nc.sbuf_tensor([128, run], f32) as T2,
        nc.sbuf_tensor([128, run], f32) as D,
        nc.sbuf_tensor([128, run], f32) as E,
        nc.sbuf_tensor([1, 1], f32) as SB11,
        nc.sbuf_tensor([128, 1], f32) as WU2,
        nc.psum_tensor([1, 1], f32) as P11,
        nc.semaphore() as dsem,
        nc.semaphore() as vsem,
        nc.semaphore() as ssem,
        nc.semaphore() as psem,
        nc.Block() as block,
    ):
        NV = 8 if small else 6  # vsem increments per iteration

        @block.sync
        def _(sync):
            for r in range(reps):
                sync.dma_start(XT[:, 0:HALF], xt[:, 0:HALF]).then_inc(dsem, 16)
                sync.wait_ge(dsem, 48 * r + 48)

        @block.scalar
        def _(scalar):
            for r in range(reps):
                if r > 0:
                    scalar.wait_ge(dsem, 48 * r)  # prior iter fully done
                # second input half on the ACT HWDGE ring, parallel with SP
                scalar.dma_start(XT[:, HALF:xtw], xt[:, HALF:xtw]).then_inc(dsem, 16)
                if small:
                    # PSUM -> SBUF -> DRAM, all on ACT (fewer cross-engine hops)
                    scalar.wait_ge(psem, r + 1)
                    scalar.copy(SB11[:], P11[:]).then_inc(ssem, 1)
                    scalar.wait_ge(ssem, r + 1)  # RAW: out-DMA reads SB11
                    scalar.dma_start(acc[:], SB11[:]).then_inc(dsem, 16)
                    continue
                if r == 0:
                    # warmup: pull the activation-table load off the critical
                    # path (runs during the input DMA; result never read)
                    scalar.square(WU2[:], nc.const_aps.tensor(0.0, (128, 1)))
                scalar.wait_ge(dsem, 48 * r + 32)
                scalar.square(X2[:], XT[:, 0:cols]).then_inc(ssem, 1)
                scalar.wait_ge(vsem, NV * r + 2)
                scalar.square(T2[:], S1[:]).then_inc(ssem, 1)
                scalar.wait_ge(psem, r + 1)
                scalar.copy(SB11[:], P11[:]).then_inc(ssem, 1)
                scalar.wait_ge(ssem, 3 * r + 3)  # RAW: out-DMA reads SB11
                scalar.dma_start(acc[:], SB11[:]).then_inc(dsem, 16)

        @block.vector
        def _(vector):
            for r in range(reps):
                vector.wait_ge(dsem, 48 * r + 32)
                if small:
                    vector.scalar_tensor_tensor(
                        X2[:], XT[:, 0:cols], 1.0, XT[:, 0:cols],
                        op0=A.mult, op1=A.mult,
                    ).then_inc(vsem, 1)  # 1
                vector.reduce_sum(
                    S1[:, 0:1], XT[:, 0:W], axis=mybir.AxisListType.X
                ).then_inc(vsem, 1)
                vector.wait_ge(vsem, NV * r + (2 if small else 1))  # RAW: initial
                vector.tensor_tensor_scan(
                    S1[:, 1:run], XT[:, W:cols], XT[:, 0 : run - 1],
                    initial=S1[:, 0:1], op0=A.add, op1=A.subtract,
                ).then_inc(vsem, 1)
                if not small:
                    vector.wait_ge(ssem, 3 * r + 1)  # X2 from ACT
                vector.reduce_sum(
                    S2[:, 0:1], X2[:, 0:W], axis=mybir.AxisListType.X
                ).then_inc(vsem, 1)
                vector.wait_ge(vsem, NV * r + (4 if small else 3))  # RAW: initial
                vector.tensor_tensor_scan(
                    S2[:, 1:run], X2[:, W:cols], X2[:, 0 : run - 1],
                    initial=S2[:, 0:1], op0=A.add, op1=A.subtract,
                ).then_inc(vsem, 1)
                if small:
                    vector.wait_ge(vsem, NV * r + 5)  # RAW: T2 reads S1/S2 path
                    vector.scalar_tensor_tensor(
                        T2[:], S1[:], 1.0, S1[:], op0=A.mult, op1=A.mult
                    ).then_inc(vsem, 1)  # 6
                    vector.wait_ge(vsem, NV * r + 6)
                else:
                    vector.wait_ge(ssem, 3 * r + 2)  # T2 from ACT
                    vector.wait_ge(vsem, NV * r + 4)  # RAW: D reads S2
                vector.scalar_tensor_tensor(
                    D[:], T2[:], -1.0 / 128.0, S2[:], op0=A.mult, op1=A.add
                ).then_inc(vsem, 1)
                vector.wait_ge(vsem, NV * r + (7 if small else 5))  # RAW: E reads D
                vector.tensor_tensor_scan(
                    E[:], XT[:, cols : cols + 1].broadcast_to([128, run]), D[:],
                    initial=0.0, op0=A.mult, op1=A.add,
                ).then_inc(vsem, 1)  # small: 8, big: 6

        @block.tensor
        def _(tensor):
            for r in range(reps):
                tensor.wait_ge(vsem, NV * r + (8 if small else 6))
                # weighted cross-partition reduce: sum_p E_last[p] * c_p
                tensor.matmul(
                    P11[:], E[:, run - 1 : run], XT[:, cols + 1 : cols + 2]
                ).then_inc(psem, 1)

    return nc


def _get_nc(run: int = RUN) -> bass.Bass:
    if run not in _NC_CACHE:
        _NC_CACHE[run] = build_nc(run=run)
    return _NC_CACHE[run]


def make_in_maps(
    x: np.ndarray, ff32: np.float32, run: int = RUN
) -> list[dict[str, np.ndarray]]:
    """Per-core input tiles covering the last 1024*run windows (all L windows
    when run=512); slot (c, p) owns windows starting at
    L - 1024*run + (c*128 + p)*run."""
    cols = run + W - 1
    start0 = L - 1024 * run
    lnff = np.log(np.float64(ff32))
    p = np.arange(128)
    in_maps = []
    for c in range(NCORES):
        base = start0 + c * 128 * run
        xt = np.empty((128, cols + 2), dtype=np.float32)
        xt[:, 0:cols] = np.lib.stride_tricks.as_strided(
            x[base:], shape=(128, cols), strides=(run * 4, 4)
        )
        xt[:, cols] = ff32
        # combine coefficient: weight of this partition's last window / 127
        i0 = L - 1 - (base + run * p + (run - 1))
        xt[:, cols + 1] = (np.exp(lnff * i0) / 127.0).astype(np.float32)
        in_maps.append({"xt": xt})
    return in_maps


def combine_host(accs: list[np.ndarray], ff32: np.float32) -> np.ndarray:
    """accs: per-core [1,1] device partial sums. Float64 host reduction."""
    ff64 = np.float64(ff32)
    total = np.float64(0.0)
    for c in range(NCORES):
        total += np.float64(np.asarray(accs[c]).reshape(()))
    norm = (1.0 - ff64) / (1.0 - np.exp(np.log(ff64) * L))
    return np.asarray(np.float32(norm * total))


def kernel(past_returns, features, raw_forgetting_factor):
    x = np.ascontiguousarray(np.asarray(past_returns, dtype=np.float32))
    assert x.shape == (N,), x.shape
    raw = np.float64(np.asarray(raw_forgetting_factor).reshape(-1)[0])
    ff32 = np.float32(1.0 / (1.0 + np.exp(-raw)))

    run = plan_run(np.float64(ff32))
    nc = _get_nc(run)
    in_maps = make_in_maps(x, ff32, run)
    res = run_bass_kernel_spmd(nc, in_maps, list(range(NCORES)))
    accs = [res.results[c]["acc"] for c in range(NCORES)]
    return combine_host(accs, ff32)



# revision 5
# speedup vs baseline: 3.0153x; 3.0153x over previous
"""EWMA predictor (sliding-window variance, exponentially weighted sum) on 8 trn2 cores.

Math: for j in [0, L): window_j = x[j : j+128], weight ff^(L-1-j),
result = norm * sum_j ff^(L-1-j) * var(window_j, ddof=1),
norm = (1-ff)/(1-ff^L), ff = sigmoid(raw_forgetting_factor).

Sharding: windows split over 8 cores x 128 partitions; partition p of core c
owns `run` consecutive windows and loads the run+127 input elements covering
them (halo overlap) plus a per-window weight row
WT[p, t] = ff^(i0(c,p) + run-1-t) / 127 in the trailing columns.

Device program (per core):
  input:  SWDGE dma_gather; row indices built on-device (Pool iotas + DVE
          int32 and/add/cast, replicated in every 16-partition group since
          each Q7 core reads the wrapped idxs from its own group), the
          descriptors prepared and triggered early so the completion
          latency overlaps the whole compute chain.
  chain:  all on DVE: X2 = x^2; one fused [128,2,W] reduce seeds both
          sliding-sum recurrences; two tensor_tensor_scan ops produce the
          128-window sums s1/s2; d = s2 - s1^2/128 (= 127*var_unbiased);
          contrib[p] = sum_t WT[p,t]*d[p,t] via tensor_tensor_reduce.
  output: SWDGE dma_scatter_add writes contrib[p] into acc[p,0] (stride
          256B rows; output DRAM buffers are zero-initialized by both the
          native runner and the bass2jax/PJRT path).
Host sums the 8x128 partials and applies norm in float64.
"""

import numpy as np

import concourse.bass as bass
import concourse.mybir as mybir
from concourse import library_config
from concourse.library_overlay import lower_extended_insts
from concourse.bass_utils import run_bass_kernel_spmd

L = 524288          # look-back windows
W = 128             # variance window length
N = L + W           # input length
NCORES = 8
WIN_PER_CORE = L // NCORES      # 65536
RUN = WIN_PER_CORE // 128       # 512 windows per partition (full computation)

_NC_CACHE = {}


def plan_run(ff64: float) -> int:
    """Windows-per-partition for the adaptive program.

    Weights ff^i are EXACTLY zero in fp32 (past subnormals) once
    i > 104/|ln ff|, so windows beyond that cannot affect any output bit.
    Keep a >=1024-window margin, round the 1024*run window count up to a
    power-of-two run, clamp to [8, 512]; run=512 is the exact full
    computation (all L windows).
    """
    lnff = np.log(np.float64(ff64))
    if not (lnff < -1e-9):
        return RUN
    k_needed = 104.0 / (-lnff)
    run_min = int(np.ceil((k_needed + 1024.0) / 1024.0))
    run = 8
    while run < run_min:
        run *= 2
    return min(run, RUN)


def geom(run: int) -> tuple[int, int]:
    cols = run + W - 1
    width = (cols + run + 63) // 64 * 64
    return cols, width


def build_nc(run: int = 8) -> bass.Bass:
    cols, width = geom(run)
    nc = bass.Bass(trn_type="TRN2")
    f32 = mybir.dt.float32
    i32 = mybir.dt.int32
    i16 = mybir.dt.int16
    A = mybir.AluOpType
    xt = nc.declare_dram_parameter("xt", [128, width], f32, isOutput=False)
    acc = nc.declare_dram_parameter("acc", [128, 64], f32, isOutput=True)

    with (
        nc.sbuf_tensor([128, 2 * width], f32) as XX,
        nc.sbuf_tensor([128, 2 * run], f32) as S,
        nc.sbuf_tensor([128, run], f32) as D,
        nc.sbuf_tensor([128, run], f32) as E,
        nc.sbuf_tensor([128, 1], f32) as CONTRIB,
        nc.sbuf_tensor([128, 8], i32) as C32,
        nc.sbuf_tensor([128, 1], i32) as P32,
        nc.sbuf_tensor([128, 8], i32) as V32,
        nc.sbuf_tensor([128, 8], i16) as IDX,
        nc.semaphore() as gsem,    # gather-in completion (DMA sem)
        nc.semaphore() as scsem,   # scatter-out completion (DMA sem)
        nc.semaphore() as psem,    # pool setup progress
        nc.semaphore() as esem,    # gather trigger enqueued
        nc.semaphore() as vsem,    # DVE progress (idx build + chain)
        nc.Block() as block,
    ):
        X = XX[:, 0:width]
        X2 = XX[:, width : width + cols]
        WT = XX[:, cols : cols + run]
        S1 = S[:, 0:run]
        S2 = S[:, run : 2 * run]

        @block.gpsimd
        def _(gpsimd):
            # Gather row index ingredients: IDX[p, c] must be (p%16) + 16c in
            # every 16-partition group (each Q7 core reads the wrapped idxs
            # from its own group; the interpreter reads group 0, hardware
            # group 1). Pool emits the affine parts, DVE the bitwise ones
            # (walrus: int bitwise ops are DVE-only, int16 alu is DVE-only).
            gpsimd.iota(
                C32[:], pattern=[[16, 8]], base=0, channel_multiplier=0
            ).then_inc(psem, 1)
            gpsimd.iota(
                P32[:], pattern=[[0, 1]], base=0, channel_multiplier=1
            ).then_inc(psem, 1)
            gpsimd.load_library(library_config.mlp)
            gpsimd.wait_ge(psem, 2)
            gpsimd.wait_ge(vsem, 3)        # IDX built by DVE
            gpsimd.dma_gather(
                X.unsqueeze(1), xt[:], IDX[:], num_idxs=128, num_idxs_reg=128,
                elem_size=width, prepare_only=True, sem=gsem,
            ).then_inc(psem, 1)
            gpsimd.wait_ge(psem, 3)
            gpsimd.trigger_dma(1)          # fire gather-in
            gpsimd.sem_inc(esem, 1)        # consumers may now wait on gsem
            gpsimd.dma_scatter_add(
                acc[0:128, 0:1], CONTRIB[:], IDX[:], num_idxs=128,
                num_idxs_reg=128, elem_size=1, elem_step=64,
                prepare_only=True, sem=scsem,
            ).then_inc(psem, 1)
            gpsimd.wait_ge(psem, 4)
            gpsimd.wait_ge(vsem, 11)       # contrib ready
            gpsimd.trigger_dma(1)          # fire scatter-out

        @block.vector
        def _(vector):
            # idx build: V32 = (P32 & 15) + C32; IDX = int16(V32)
            vector.wait_ge(psem, 2)
            vector.tensor_scalar(
                P32[:], P32[:], 15, None, op0=A.bitwise_and
            ).then_inc(vsem, 1)
            vector.wait_ge(vsem, 1)
            vector.tensor_tensor(
                V32[:], C32[:], P32[:].broadcast_to([128, 8]), op=A.add
            ).then_inc(vsem, 1)
            vector.wait_ge(vsem, 2)
            vector.tensor_copy(IDX[:], V32[:]).then_inc(vsem, 1)

            # compute chain
            vector.wait_ge(esem, 1)
            vector.wait_ge(gsem, 16)
            vector.scalar_tensor_tensor(
                X2[:], X[:, 0:cols], 1.0, X[:, 0:cols], op0=A.mult, op1=A.mult
            ).then_inc(vsem, 1)
            vector.wait_ge(vsem, 4)
            # fused initial sums: S[:, 0] = sum x[0:W], S[:, run] = sum x2[0:W]
            vector.reduce_sum(
                S[:].rearrange("p (g r) -> p g r", g=2)[:, :, 0:1],
                XX[:].rearrange("p (g c) -> p g c", g=2)[:, :, 0:W],
                axis=mybir.AxisListType.X,
            ).then_inc(vsem, 1)
            vector.wait_ge(vsem, 5)
            # sliding-sum scans: s[t] = (x[t+W-1] + s[t-1]) - x[t-1]
            vector.tensor_tensor_scan(
                S1[:, 1:run], X[:, W:cols], X[:, 0 : run - 1],
                initial=S1[:, 0:1], op0=A.add, op1=A.subtract,
            ).then_inc(vsem, 1)
            vector.wait_ge(vsem, 6)
            vector.tensor_tensor_scan(
                S2[:, 1:run], X2[:, W:cols], XX[:, width : width + run - 1],
                initial=S2[:, 0:1], op0=A.add, op1=A.subtract,
            ).then_inc(vsem, 1)
            vector.wait_ge(vsem, 7)
            # d = s2 - s1^2/128  (D = (s1 * -1/128) * s1; D = D + s2)
            vector.scalar_tensor_tensor(
                D[:], S1[:], -1.0 / 128.0, S1[:], op0=A.mult, op1=A.mult
            ).then_inc(vsem, 1)
            vector.wait_ge(vsem, 8)
            vector.scalar_tensor_tensor(
                D[:], D[:], 1.0, S2[:], op0=A.mult, op1=A.add
            ).then_inc(vsem, 1)
            vector.wait_ge(vsem, 9)
            # contrib[p] = sum_t WT[p,t] * d[p,t]
            vector.scalar_tensor_tensor(
                E[:], D[:], 1.0, WT[:], op0=A.mult, op1=A.mult
            ).then_inc(vsem, 1)
            vector.wait_ge(vsem, 10)
            vector.reduce_sum(
                CONTRIB[:], E[:], axis=mybir.AxisListType.X
            ).then_inc(vsem, 1)

    lower_extended_insts(nc)  # encode ISA bytes for the NEFF compiler
    return nc


def _get_nc(run: int) -> bass.Bass:
    if run not in _NC_CACHE:
        _NC_CACHE[run] = build_nc(run)
    return _NC_CACHE[run]


def make_in_maps(
    x: np.ndarray, ff32: np.float32, run: int
) -> list[dict[str, np.ndarray]]:
    """Per-core input tiles covering the last 1024*run windows (all L windows
    when run=512); slot (c, p) owns windows starting at
    L - 1024*run + (c*128 + p)*run."""
    cols, width = geom(run)
    start0 = L - 1024 * run
    lnff = np.log(np.float64(ff32))
    p = np.arange(128)
    t = np.arange(run)
    in_maps = []
    for c in range(NCORES):
        base = start0 + c * 128 * run
        xt = np.zeros((128, width), dtype=np.float32)
        xt[:, 0:cols] = np.lib.stride_tricks.as_strided(
            x[base:], shape=(128, cols), strides=(run * 4, 4)
        )
        # weight of window t of partition p: global index i = i0 + run-1-t
        i0 = L - 1 - (base + run * p + (run - 1))
        expo = i0[:, None] + (run - 1 - t)[None, :]
        xt[:, cols : cols + run] = (np.exp(lnff * expo) / 127.0).astype(
            np.float32
        )
        in_maps.append({"xt": xt})
    return in_maps


def combine_host(accs: list[np.ndarray], ff32: np.float32) -> np.ndarray:
    """accs: per-core [128,64] tiles, partial sums in column 0. f64 host sum."""
    ff64 = np.float64(ff32)
    total = np.float64(0.0)
    for c in range(NCORES):
        total += np.asarray(accs[c])[:, 0].astype(np.float64).sum()
    norm = (1.0 - ff64) / (1.0 - np.exp(np.log(ff64) * L))
    return np.asarray(np.float32(norm * total))


def kernel(past_returns, features, raw_forgetting_factor):
    x = np.ascontiguousarray(np.asarray(past_returns, dtype=np.float32))
    assert x.shape == (N,), x.shape
    raw = np.float64(np.asarray(raw_forgetting_factor).reshape(-1)[0])
    ff32 = np.float32(1.0 / (1.0 + np.exp(-raw)))

    run = plan_run(np.float64(ff32))
    nc = _get_nc(run)
    in_maps = make_in_maps(x, ff32, run)
    res = run_bass_kernel_spmd(nc, in_maps, list(range(NCORES)))
    accs = [res.results[c]["acc"] for c in range(NCORES)]
    return combine_host(accs, ff32)


# revision 6
# speedup vs baseline: 3.3940x; 1.1256x over previous
"""EWMA predictor (sliding-window variance, exponentially weighted sum) on 8 trn2 cores.

Math: for j in [0, L): window_j = x[j : j+128], weight ff^(L-1-j),
result = norm * sum_j ff^(L-1-j) * var(window_j, ddof=1),
norm = (1-ff)/(1-ff^L), ff = sigmoid(raw_forgetting_factor).

Sharding: windows split over 8 cores x 128 partitions; partition p of core c
owns `run` consecutive windows and loads the run+127 input elements covering
them (halo overlap) plus a per-window weight row
WT[p, t] = ff^(i0(c,p) + run-1-t) / 127 in the trailing columns.

Device program (per core):
  input:  SWDGE dma_gather; row indices built on-device (Pool iotas + DVE
          int32 and/add/cast, replicated in every 16-partition group since
          each Q7 core reads the wrapped idxs from its own group), the
          descriptors prepared and triggered early so the completion
          latency overlaps the whole compute chain.
  chain:  all on DVE: X2 = x^2; one fused [128,2,W] reduce seeds both
          sliding-sum recurrences; two tensor_tensor_scan ops produce the
          128-window sums s1/s2; d = s2 - s1^2/128 (= 127*var_unbiased);
          contrib[p] = sum_t WT[p,t]*d[p,t] via tensor_tensor_reduce.
  output: SWDGE dma_scatter_add writes contrib[p] into acc[p,0] (stride
          256B rows; output DRAM buffers are zero-initialized by both the
          native runner and the bass2jax/PJRT path).
Host sums the 8x128 partials and applies norm in float64.
"""

import numpy as np

import concourse.bass as bass
import concourse.mybir as mybir
from concourse import library_config
from concourse.library_overlay import lower_extended_insts
from concourse.bass_utils import run_bass_kernel_spmd

L = 524288          # look-back windows
W = 128             # variance window length
N = L + W           # input length
NCORES = 8
WIN_PER_CORE = L // NCORES      # 65536
RUN = WIN_PER_CORE // 128       # 512 windows per partition (full computation)

_NC_CACHE = {}


def plan_run(ff64: float) -> int:
    """Windows-per-partition for the adaptive program.

    Weights ff^i are EXACTLY zero in fp32 (past subnormals) once
    i > 104/|ln ff|, so windows beyond that cannot affect any output bit.
    Keep a >=1024-window margin, round the 1024*run window count up to a
    power-of-two run, clamp to [8, 512]; run=512 is the exact full
    computation (all L windows).
    """
    lnff = np.log(np.float64(ff64))
    if not (lnff < -1e-9):
        return RUN
    k_needed = 104.0 / (-lnff)
    run_min = int(np.ceil((k_needed + 1024.0) / 1024.0))
    run = 8
    while run < run_min:
        run *= 2
    return min(run, RUN)


def geom(run: int) -> tuple[int, int]:
    cols = run + W - 1
    width = (cols + run + 63) // 64 * 64
    return cols, width


def build_nc(run: int = 8) -> bass.Bass:
    cols, width = geom(run)
    nc = bass.Bass(trn_type="TRN2")
    f32 = mybir.dt.float32
    i32 = mybir.dt.int32
    i16 = mybir.dt.int16
    A = mybir.AluOpType
    xt = nc.declare_dram_parameter("xt", [128, width], f32, isOutput=False)
    acc = nc.declare_dram_parameter("acc", [128, 64], f32, isOutput=True)

    with (
        nc.sbuf_tensor([128, 2 * width], f32) as XX,
        nc.sbuf_tensor([128, 2 * run], f32) as S,
        nc.sbuf_tensor([128, run], f32) as D,
        nc.sbuf_tensor([128, run], f32) as E,
        nc.sbuf_tensor([128, 1], f32) as CONTRIB,
        nc.sbuf_tensor([128, 8], i32) as C32,
        nc.sbuf_tensor([128, 1], i32) as P32,
        nc.sbuf_tensor([128, 8], i32) as V32,
        nc.sbuf_tensor([128, 8], i16) as IDX,
        nc.sbuf_tensor([128, 150], f32) as DLY,
        nc.sbuf_tensor([1, 128], f32) as PDLY,
        nc.semaphore() as gsem,    # gather-in completion (DMA sem)
        nc.semaphore() as scsem,   # scatter-out completion (DMA sem)
        nc.semaphore() as psem,    # pool setup progress
        nc.semaphore() as esem,    # gather trigger enqueued
        nc.semaphore() as vsem,    # DVE progress (idx build + chain)
        nc.Block() as block,
    ):
        X = XX[:, 0:width]
        X2 = XX[:, width : width + cols]
        WT = XX[:, cols : cols + run]
        S1 = S[:, 0:run]
        S2 = S[:, run : 2 * run]

        @block.gpsimd
        def _(gpsimd):
            # Gather row index ingredients: IDX[p, c] must be (p%16) + 16c in
            # every 16-partition group (each Q7 core reads the wrapped idxs
            # from its own group; the interpreter reads group 0, hardware
            # group 1). Pool emits the affine parts, DVE the bitwise ones
            # (walrus: int bitwise ops are DVE-only, int16 alu is DVE-only).
            gpsimd.iota(
                C32[:], pattern=[[16, 8]], base=0, channel_multiplier=0
            ).then_inc(psem, 1)
            gpsimd.iota(
                P32[:], pattern=[[0, 1]], base=0, channel_multiplier=1
            ).then_inc(psem, 1)
            gpsimd.memset(PDLY[:, 0:64], 0)
            gpsimd.memset(PDLY[:, 64:128], 0)
            gpsimd.wait_ge(psem, 2)
            gpsimd.wait_ge(vsem, 1)        # P32 &= 15 done on DVE
            gpsimd.tensor_tensor(
                V32[:], C32[:], P32[:].broadcast_to([128, 8]), op=A.add
            ).then_inc(psem, 1)
            gpsimd.wait_ge(psem, 3)
            gpsimd.tensor_copy(IDX[:], V32[:]).then_inc(psem, 1)
            gpsimd.load_library(library_config.mlp)
            gpsimd.wait_ge(psem, 4)
            gpsimd.dma_gather(
                X.unsqueeze(1), xt[:], IDX[:], num_idxs=128, num_idxs_reg=128,
                elem_size=width, prepare_only=True, sem=gsem,
            ).then_inc(psem, 1)
            gpsimd.wait_ge(psem, 5)
            gpsimd.trigger_dma(1)          # fire gather-in
            gpsimd.sem_inc(esem, 1)        # consumers may now wait on gsem
            gpsimd.dma_scatter_add(
                acc[0:128, 0:1], CONTRIB[:], IDX[:], num_idxs=128,
                num_idxs_reg=128, elem_size=1, elem_step=64,
                prepare_only=True, sem=scsem,
            ).then_inc(psem, 1)
            gpsimd.wait_ge(psem, 6)
            gpsimd.wait_ge(vsem, 9)        # contrib ready
            gpsimd.trigger_dma(1)          # fire scatter-out
            gpsimd.wait_ge(scsem, 16)      # real-HW: scatter landed before exit

        @block.vector
        def _(vector):
            # idx build step 1 (bitwise is DVE-only): P32 &= 15
            vector.wait_ge(psem, 2)
            vector.tensor_scalar(
                P32[:], P32[:], 15, None, op0=A.bitwise_and
            ).then_inc(vsem, 1)
            # self-delay sized to arrive at the waits just after the Pool
            # trigger enqueues the gather (eager sem pass); if the trigger is
            # later than this, the blocked wait wakes normally (+100ns).
            vector.memset(DLY[:], 0.0)

            # compute chain
            vector.wait_ge(esem, 1)
            vector.wait_ge(gsem, 16)
            vector.scalar_tensor_tensor(
                X2[:], X[:, 0:cols], 1.0, X[:, 0:cols], op0=A.mult, op1=A.mult
            ).then_inc(vsem, 1)
            vector.wait_ge(vsem, 2)
            # fused initial sums: S[:, 0] = sum x[0:W], S[:, run] = sum x2[0:W]
            vector.reduce_sum(
                S[:].rearrange("p (g r) -> p g r", g=2)[:, :, 0:1],
                XX[:].rearrange("p (g c) -> p g c", g=2)[:, :, 0:W],
                axis=mybir.AxisListType.X,
            ).then_inc(vsem, 1)
            vector.wait_ge(vsem, 3)
            # sliding-sum scans: s[t] = (x[t+W-1] + s[t-1]) - x[t-1]
            vector.tensor_tensor_scan(
                S1[:, 1:run], X[:, W:cols], X[:, 0 : run - 1],
                initial=S1[:, 0:1], op0=A.add, op1=A.subtract,
            ).then_inc(vsem, 1)
            vector.wait_ge(vsem, 4)
            vector.tensor_tensor_scan(
                S2[:, 1:run], X2[:, W:cols], XX[:, width : width + run - 1],
                initial=S2[:, 0:1], op0=A.add, op1=A.subtract,
            ).then_inc(vsem, 1)
            vector.wait_ge(vsem, 5)
            # d = s2 - s1^2/128  (D = (s1 * -1/128) * s1; D = D + s2)
            vector.scalar_tensor_tensor(
                D[:], S1[:], -1.0 / 128.0, S1[:], op0=A.mult, op1=A.mult
            ).then_inc(vsem, 1)
            vector.wait_ge(vsem, 6)
            vector.scalar_tensor_tensor(
                D[:], D[:], 1.0, S2[:], op0=A.mult, op1=A.add
            ).then_inc(vsem, 1)
            vector.wait_ge(vsem, 7)
            # contrib[p] = sum_t WT[p,t] * d[p,t]
            vector.scalar_tensor_tensor(
                E[:], D[:], 1.0, WT[:], op0=A.mult, op1=A.mult
            ).then_inc(vsem, 1)
            vector.wait_ge(vsem, 8)
            vector.reduce_sum(
                CONTRIB[:], E[:], axis=mybir.AxisListType.X
            ).then_inc(vsem, 1)

    lower_extended_insts(nc)  # encode ISA bytes for the NEFF compiler
    return nc


def _get_nc(run: int) -> bass.Bass:
    if run not in _NC_CACHE:
        _NC_CACHE[run] = build_nc(run)
    return _NC_CACHE[run]


def make_in_maps(
    x: np.ndarray, ff32: np.float32, run: int
) -> list[dict[str, np.ndarray]]:
    """Per-core input tiles covering the last 1024*run windows (all L windows
    when run=512); slot (c, p) owns windows starting at
    L - 1024*run + (c*128 + p)*run."""
    cols, width = geom(run)
    start0 = L - 1024 * run
    lnff = np.log(np.float64(ff32))
    p = np.arange(128)
    t = np.arange(run)
    in_maps = []
    for c in range(NCORES):
        base = start0 + c * 128 * run
        xt = np.zeros((128, width), dtype=np.float32)
        xt[:, 0:cols] = np.lib.stride_tricks.as_strided(
            x[base:], shape=(128, cols), strides=(run * 4, 4)
        )
        # weight of window t of partition p: global index i = i0 + run-1-t
        i0 = L - 1 - (base + run * p + (run - 1))
        expo = i0[:, None] + (run - 1 - t)[None, :]
        xt[:, cols : cols + run] = (np.exp(lnff * expo) / 127.0).astype(
            np.float32
        )
        in_maps.append({"xt": xt})
    return in_maps


def combine_host(accs: list[np.ndarray], ff32: np.float32) -> np.ndarray:
    """accs: per-core [128,64] tiles, partial sums in column 0. f64 host sum."""
    ff64 = np.float64(ff32)
    total = np.float64(0.0)
    for c in range(NCORES):
        total += np.asarray(accs[c])[:, 0].astype(np.float64).sum()
    norm = (1.0 - ff64) / (1.0 - np.exp(np.log(ff64) * L))
    return np.asarray(np.float32(norm * total))


def kernel(past_returns, features, raw_forgetting_factor):
    x = np.ascontiguousarray(np.asarray(past_returns, dtype=np.float32))
    assert x.shape == (N,), x.shape
    raw = np.float64(np.asarray(raw_forgetting_factor).reshape(-1)[0])
    ff32 = np.float32(1.0 / (1.0 + np.exp(-raw)))

    run = plan_run(np.float64(ff32))
    nc = _get_nc(run)
    in_maps = make_in_maps(x, ff32, run)
    res = run_bass_kernel_spmd(nc, in_maps, list(range(NCORES)))
    accs = [res.results[c]["acc"] for c in range(NCORES)]
    return combine_host(accs, ff32)
